# revision 1
# baseline (speedup 1.0000x reference)
"""Trainium2 Bass kernel for nn_Grid_fun: out = tile(feat(z), 6) @ a.

Math: z = [x, 1] (N,4); feat = (z⊗z).reshape(N,16); out = tile(feat,6) @ a
    = feat @ a_eff  where a_eff = a.reshape(6,16,3).sum(0)   [16,3]
    => out[n,c] = z[n]^T A_c z[n],  A_c = a_eff[:,c].reshape(4,4)

Device algorithm (per core, data-parallel over N):
  Host stages x as Z[3g+j, m] = x[12 m + g, j]  (12 groups x 3 comps = 36
  partition rows, points along the free dim).
  mm1:  V[108,F] = P_V^T @ Z       (9 linear forms per group)
  ACT:  R = Square(V + bias)       (basis {X^2,Y^2,Z^2,(X+Y)^2,(X+Z)^2,
                                    (Y+Z)^2,(u_c.x+1)^2 c=0..2})
  mm2:  O[36,F] = A_blk^T @ R      (block-diag 9->3 per group), written at
        PSUM base partition 0 / 64 for alternating column tiles
  DVE:  out_sb = O + k_vec         (folds the constant term), DMA out.
The square basis exactly reproduces the quadratic + linear + constant parts:
  quad: 6 canonical squares; linear: w_c*(u_c.x+1)^2 with u_c = L_c/(2 k_c)
  (quadratic pollution subtracted via the canonical basis); const: k_c folded
  into the output copy.
"""

import sys

if "/opt/trn_rl_repo" not in sys.path:
    sys.path.insert(0, "/opt/trn_rl_repo")

from contextlib import ExitStack

import numpy as np

import concourse.bass as bass
import concourse.mybir as mybir
import concourse.tile as tile
from concourse import bacc
from concourse.bass_utils import run_bass_kernel_spmd

N_CORES = 8
N_POINTS = 1_000_000
N_PER_CORE = N_POINTS // N_CORES  # 125000
G = 12  # points (groups) per column
FTOT = 10418  # columns per core; G*FTOT = 125016 >= N_PER_CORE
NPAD = G * FTOT
FT = 512  # matmul free-dim tile
NTILES = (FTOT + 2 * FT - 1) // (2 * FT)  # macro tiles of 2*FT columns

_CACHE: dict = {}


def _build_nc():
    nc = bacc.Bacc("TRN2", target_bir_lowering=False)
    f32 = mybir.dt.float32

    z_d = nc.dram_tensor("z", [36, FTOT], f32, kind="ExternalInput")
    pv_d = nc.dram_tensor("pv", [36, 108], f32, kind="ExternalInput")
    ab_d = nc.dram_tensor("ab", [108, 36], f32, kind="ExternalInput")
    bias_d = nc.dram_tensor("bias", [108, 1], f32, kind="ExternalInput")
    kv_d = nc.dram_tensor("kv", [128, 1], f32, kind="ExternalInput")
    o_d = nc.dram_tensor("o", [72, FTOT // 2], f32, kind="ExternalOutput")

    with tile.TileContext(nc) as tc:
        with ExitStack() as ctx:
            cpool = ctx.enter_context(tc.tile_pool(name="consts", bufs=1))
            zpool = ctx.enter_context(tc.tile_pool(name="zt", bufs=3))
            rpool = ctx.enter_context(tc.tile_pool(name="rt", bufs=2))
            opool = ctx.enter_context(tc.tile_pool(name="ot", bufs=3))
            vpool = ctx.enter_context(
                tc.tile_pool(name="vps", bufs=2, space="PSUM")
            )
            ops_pool = ctx.enter_context(
                tc.tile_pool(name="ops", bufs=2, space="PSUM")
            )

            pv = cpool.tile([36, 108], f32)
            nc.gpsimd.dma_start(pv[:], pv_d[:, :])
            ab = cpool.tile([108, 36], f32)
            nc.gpsimd.dma_start(ab[:], ab_d[:, :])
            bias = cpool.tile([108, 1], f32)
            nc.gpsimd.dma_start(bias[:], bias_d[:, :])
            kv = cpool.tile([128, 1], f32)
            nc.gpsimd.dma_start(kv[:], kv_d[:, :])

            for t in range(NTILES):
                c0 = 2 * FT * t
                w = min(2 * FT, FTOT - c0)  # macro width (2*FT or tail)
                h = w // 2
                zt = zpool.tile([36, 2 * FT], f32)
                nc.sync.dma_start(zt[:, :w], z_d[:, c0 : c0 + w])

                vps = vpool.tile([108, 2 * FT], f32)
                nc.tensor.matmul(
                    vps[:, :h], pv[:], zt[:, :h], start=True, stop=True
                )
                nc.tensor.matmul(
                    vps[:, h:w], pv[:], zt[:, h:w], start=True, stop=True
                )

                rt = rpool.tile([108, 2 * FT], f32)
                nc.scalar.activation(
                    rt[:, :w],
                    vps[:, :w],
                    mybir.ActivationFunctionType.Square,
                    bias=bias[:],
                    scale=1.0,
                )

                ops = ops_pool.tile([128, FT], f32)
                nc.tensor.matmul(
                    ops[0:36, :h], ab[:], rt[:, :h], start=True, stop=True
                )
                nc.tensor.matmul(
                    ops[64:100, :h], ab[:], rt[:, h:w], start=True, stop=True
                )

                ot = opool.tile([128, FT], f32)
                nc.vector.tensor_scalar(
                    ot[0:100, :h],
                    ops[0:100, :h],
                    kv[0:100],
                    None,
                    mybir.AluOpType.add,
                )
                oc = c0 // 2
                nc.sync.dma_start(o_d[0:36, oc : oc + h], ot[0:36, :h])
                nc.sync.dma_start(o_d[36:72, oc : oc + h], ot[64:100, :h])
    nc.compile()
    return nc


def _coeffs(a: np.ndarray):
    """Host-side prep of the constant matrices from param a [96,3]."""
    a_eff = a.reshape(6, 16, 3).sum(0)  # [16,3]
    A = a_eff.T.reshape(3, 4, 4)  # A[c] with out_c = z^T A_c z
    As = 0.5 * (A + A.transpose(0, 2, 1))  # symmetrize
    Q = As[:, :3, :3]  # [3,3,3] quadratic part
    L = 2.0 * As[:, :3, 3]  # [3,3] linear coefs
    K = As[:, 3, 3].copy()  # [3] constants
    # guard tiny K (u_c = L_c / (2 K_c)); shift the constant via kv fold
    Ksafe = np.where(np.abs(K) < 1e-3, 1.0, K)
    U = L / (2.0 * Ksafe[:, None])  # [3,3] tailored directions

    # basis quadratic parts: M[s] (3x3 sym) for s=0..8
    E = np.eye(3, dtype=np.float64)
    dirs = [
        (E[0], E[0]), (E[1], E[1]), (E[2], E[2]),
        (E[0] + E[1], E[0] + E[1]),
        (E[0] + E[2], E[0] + E[2]),
        (E[1] + E[2], E[1] + E[2]),
    ]
    M = np.zeros((9, 3, 3))
    for s, (u, v) in enumerate(dirs):
        M[s] = np.outer(u, v)
    for c in range(3):
        M[6 + c] = np.outer(U[c], U[c])
    # solve for weights: Q[c] = sum_s w[c,s] M[s] with constraints:
    # w[c,6+c'] = Ksafe[c] if c'==c else 0  (the tailored square carries
    # the linear term with weight K so 2*w*u = L)
    Mflat = M.reshape(9, 9)[:, [0, 1, 2, 4, 5, 8, 1, 2, 5]]
    # use upper-tri representation: entries (00,11,22,01,02,12) with
    # off-diag doubled
    def sym6(S):
        return np.array(
            [S[0, 0], S[1, 1], S[2, 2], S[0, 1] + S[1, 0],
             S[0, 2] + S[2, 0], S[1, 2] + S[2, 1]]
        )

    B6 = np.stack([sym6(M[s]) for s in range(9)])  # [9,6]
    W = np.zeros((3, 9))
    for c in range(3):
        rhs = sym6(Q[c]) - Ksafe[c] * B6[6 + c]
        W[c, :6] = np.linalg.solve(B6[:6].T, rhs)
        W[c, 6 + c] = Ksafe[c]
    # constant leftover: out_c = sum_s W[c,s] q_s + kconst[c]
    # tailored square contributes Ksafe*1 at x=0... full check:
    # value at x=0: sum_s W[c,s]*(bias_s)^2 = W[c,6+c]*1 = Ksafe[c]
    kconst = K - Ksafe
    return U, W, kconst


def _host_tensors(a: np.ndarray):
    U, W, kconst = _coeffs(a.astype(np.float64))
    pv = np.zeros((36, 108), dtype=np.float32)
    bias = np.zeros((108, 1), dtype=np.float32)
    ab = np.zeros((108, 36), dtype=np.float32)
    kv = np.zeros((128, 1), dtype=np.float32)
    forms = [
        [(0, 1.0)], [(1, 1.0)], [(2, 1.0)],
        [(0, 1.0), (1, 1.0)], [(0, 1.0), (2, 1.0)], [(1, 1.0), (2, 1.0)],
    ]
    for g in range(G):
        for s in range(9):
            col = 9 * g + s
            if s < 6:
                for j, v in forms[s]:
                    pv[3 * g + j, col] = v
            else:
                c = s - 6
                for j in range(3):
                    pv[3 * g + j, col] = U[c, j]
                bias[col, 0] = 1.0
        for c in range(3):
            orow = 3 * g + c
            for s in range(9):
                ab[9 * g + s, orow] = W[c, s]
    for g in range(G):
        for c in range(3):
            kv[3 * g + c, 0] = kconst[c]
            kv[64 + 3 * g + c, 0] = kconst[c]
    return pv, ab, bias, kv


def kernel(x: np.ndarray, a: np.ndarray) -> np.ndarray:
    x = np.ascontiguousarray(x, dtype=np.float32)
    a = np.ascontiguousarray(a, dtype=np.float32)
    if "nc" not in _CACHE:
        _CACHE["nc"] = _build_nc()
    nc = _CACHE["nc"]

    pv, ab, bias, kv = _host_tensors(a)
    in_maps = []
    for ci in range(N_CORES):
        xs = x[ci * N_PER_CORE : (ci + 1) * N_PER_CORE]
        xp = np.zeros((NPAD, 3), dtype=np.float32)
        xp[:N_PER_CORE] = xs
        z = np.ascontiguousarray(
            xp.reshape(FTOT, G, 3).transpose(1, 2, 0).reshape(36, FTOT)
        )
        in_maps.append({"z": z, "pv": pv, "ab": ab, "bias": bias, "kv": kv})

    res = run_bass_kernel_spmd(nc, in_maps, list(range(N_CORES)))

    out = np.empty((N_POINTS, 3), dtype=np.float32)
    H = FT  # half-macro width
    for ci in range(N_CORES):
        o = res.results[ci]["o"]  # [72, FTOT//2]
        full = np.empty((NPAD, 3), dtype=np.float32)
        # column m of Z maps: rows[3g+c] of half h -> point 12*m_global+g
        # macro t covers Z cols [2*FT*t, 2*FT*t+w); half0 -> o rows 0:36 at
        # o-cols [FT*t ...], half1 -> o rows 36:72
        ov = o.reshape(2, G, 3, FTOT // 2)  # [half_rows, g, c, ocol]
        ncols_half = FTOT // 2
        # Build m_global for each (half, ocol): m = 2*FT*t + h*half_w + k
        # where ocol = FT*t + k, half_w = w//2. For full tiles half_w = FT.
        # Tail tile (w < 2*FT) also has half_w = w//2 = h_tail and its ocols
        # span [FT*t, FT*t + h_tail).
        mcols = np.empty((2, ncols_half), dtype=np.int64)
        for t in range(NTILES):
            c0 = 2 * FT * t
            w = min(2 * FT, FTOT - c0)
            h = w // 2
            oc = c0 // 2
            k = np.arange(h)
            mcols[0, oc : oc + h] = c0 + k
            mcols[1, oc : oc + h] = c0 + h + k
        for half in range(2):
            m = mcols[half]  # [ncols_half]
            pts = (G * m[:, None] + np.arange(G)[None, :]).ravel()  # [ncols*G]
            vals = ov[half].transpose(2, 0, 1).reshape(ncols_half * G, 3)
            full[pts] = vals
        out[ci * N_PER_CORE : (ci + 1) * N_PER_CORE] = full[:N_PER_CORE]
    return out



# revision 9
# speedup vs baseline: 1.1181x; 1.1181x over previous
"""Trainium2 Bass kernel for nn_Grid_fun: out = tile(feat(z), 6) @ a.

Math: z = [x, 1] (N,4); feat = (z otimes z).reshape(N,16); out = tile(feat,6) @ a
    = feat @ a_eff  where a_eff = a.reshape(6,16,3).sum(0)   [16,3]
    => out[n,c] = z[n]^T A_c z[n],  A_c = a_eff[:,c].reshape(4,4)

Device algorithm (per core, data-parallel over N):
  Host stages x as Z2 [106, 4608] bf16: half h at partition base 64h holds
  rows 3g+j of z-cols m = 4608h + u (G=14 points per z-col, F=9216 z-cols).
  PE base partitions must be in {0,64} for K,M<=64 - hence the two-half
  layout (~83% DMA partition utilization, bf16 halves the line bytes).
  Per vps tile v (6 total; h=v%2, col-group jg=v//2):
    mm1 x3: V[127, 512i..] = pv^T @ Z2[64h:64h+42, 512(3jg+i)..]   (bf16)
    ACT:    R[127, 1536] = Square(V + bias)  (canonical/tailored square
            basis; R row 126 = (0+1)^2 = 1 carries the constant via ab)
    mm2 x3: block k=3v+i -> pps[k//2][64*(k%2) : +42] = ab^T @ R  (fp32r,
            2-stacked in PSUM at offsets {0,64})
  DVE copies pps -> bf16 SBUF (junk rows 42:64 ignored by host), gpsimd
  issues the output DMAs. PE warm-up matmuls fight the HAM clock gate.
"""

import sys

if "/opt/trn_rl_repo" not in sys.path:
    sys.path.insert(0, "/opt/trn_rl_repo")

from contextlib import ExitStack

import ml_dtypes
import numpy as np

import concourse.bass as bass
import concourse.mybir as mybir
import concourse.tile as tile
from concourse import bacc
from concourse.bass_utils import run_bass_kernel_spmd

N_CORES = 8
N_POINTS = 1_000_000
N_PER_CORE = N_POINTS // N_CORES  # 125000
G = 14  # points per z-column
F = 9216  # z-columns per core (18*512); G*F = 129024 >= N_PER_CORE
NPAD = G * F
HCOLS = F // 2  # 4608 z-cols per half
NV = 6  # vps tiles (3 blocks of 512 each)
CH = 512
N_WARM = 12  # PE warm-up matmuls

_CACHE: dict = {}


def _build_nc():
    nc = bacc.Bacc("TRN2", target_bir_lowering=False)
    f32 = mybir.dt.float32
    f32r = mybir.dt.float32r
    bf16 = mybir.dt.bfloat16

    z_d = nc.dram_tensor("z", [106, HCOLS], bf16, kind="ExternalInput")
    pv_d = nc.dram_tensor("pv", [106, 127], bf16, kind="ExternalInput")
    ab_d = nc.dram_tensor("ab", [127, 42], bf16, kind="ExternalInput")
    bias_d = nc.dram_tensor("bias", [127, 1], f32, kind="ExternalInput")
    o_d = nc.dram_tensor("o", [106, HCOLS], bf16, kind="ExternalOutput")
    sink_d = nc.dram_tensor("sink", [127, 2], f32, kind="ExternalOutput")

    with tile.TileContext(nc) as tc:
        with ExitStack() as ctx:
            cpool = ctx.enter_context(tc.tile_pool(name="consts", bufs=1))
            zpool = ctx.enter_context(tc.tile_pool(name="zt", bufs=2))
            rpool = ctx.enter_context(tc.tile_pool(name="rt", bufs=2))
            opool = ctx.enter_context(tc.tile_pool(name="ot", bufs=2))
            vpool = ctx.enter_context(
                tc.tile_pool(name="vps", bufs=2, space="PSUM")
            )
            ppool = ctx.enter_context(
                tc.tile_pool(name="pps", bufs=2, space="PSUM")
            )

            pv = cpool.tile([106, 127], bf16)
            nc.sync.dma_start(pv[:], pv_d[:, :])
            ab = cpool.tile([127, 42], bf16)
            nc.sync.dma_start(ab[:], ab_d[:, :])
            bias = cpool.tile([127, 1], f32)
            nc.sync.dma_start(bias[:], bias_d[:, :])

            # ACT table warm-up (Square table load ~1.3us) off critical path:
            # depends only on the tiny bias DMA, so it runs at ~0.3us.
            wsq = cpool.tile([127, 2], f32)
            nc.scalar.activation(
                wsq[:, 0:1], bias[:], mybir.ActivationFunctionType.Square,
                bias=bias[:], scale=1.0,
            )
            # PE warm-up: HAM un-throttles after ~3.4us of sustained PE
            # activity; burn the initial DMA-wait advancing that window.
            warm = ppool.tile([127, CH], f32, tag="pps")
            for _ in range(N_WARM):
                nc.tensor.matmul(
                    warm[:, 0:127], pv[0:42, :], pv[0:42, 0:127],
                    start=True, stop=True,
                )
            # Keep the warm-ups live (read one column into the sink output).
            nc.vector.tensor_copy(wsq[:, 1:2], warm[:, 0:1])
            nc.sync.dma_start(sink_d[:, :], wsq[:])

            pps_tiles = {}
            for v in range(NV):
                h, jg = v % 2, v // 2
                if h == 0:
                    zt = zpool.tile([106, 3 * CH], bf16)
                    nc.sync.dma_start(
                        zt[:], z_d[:, 3 * CH * jg : 3 * CH * (jg + 1)]
                    )

                vps = vpool.tile([127, 3 * CH], f32)
                for i in range(3):
                    nc.tensor.matmul(
                        vps[:, i * CH : (i + 1) * CH],
                        pv[64 * h : 64 * h + 42, :],
                        zt[64 * h : 64 * h + 42, i * CH : (i + 1) * CH],
                        start=True,
                        stop=True,
                    )

                rt = rpool.tile([127, 3 * CH], bf16)
                nc.scalar.activation(
                    rt[:],
                    vps[:],
                    mybir.ActivationFunctionType.Square,
                    bias=bias[:],
                    scale=1.0,
                )
                for i in range(3):
                    k = 3 * v + i
                    p, s = k // 2, k % 2
                    if s == 0:
                        pps_tiles[p] = ppool.tile(
                            [106, CH], f32, tag="pps", name=f"pps{p}"
                        )
                    nc.tensor.matmul(
                        pps_tiles[p][64 * s : 64 * s + 42, :],
                        ab[:],
                        rt[:, i * CH : (i + 1) * CH],
                        start=True,
                        stop=True,
                    )
                    if s == 1:
                        if p % 3 == 0:
                            ot = opool.tile([106, 3 * CH], bf16)
                        nc.vector.tensor_copy(
                            ot[:, (p % 3) * CH : (p % 3 + 1) * CH],
                            pps_tiles[p][:],
                        )
                        if p % 3 == 2:
                            oc = (p - 2) * CH
                            nc.gpsimd.dma_start(
                                o_d[:, oc : oc + 3 * CH], ot[:]
                            )
    nc.compile()
    return nc


def _coeffs(a: np.ndarray):
    """Host-side prep of the constant matrices from param a [96,3]."""
    a_eff = a.reshape(6, 16, 3).sum(0)  # [16,3]
    A = a_eff.T.reshape(3, 4, 4)  # A[c] with out_c = z^T A_c z
    As = 0.5 * (A + A.transpose(0, 2, 1))  # symmetrize
    Q = As[:, :3, :3]  # [3,3,3] quadratic part
    L = 2.0 * As[:, :3, 3]  # [3,3] linear coefs
    K = As[:, 3, 3].copy()  # [3] constants
    # guard tiny K (u_c = L_c / (2 K_c)); shift the constant via kconst fold
    Ksafe = np.where(np.abs(K) < 1e-3, 1.0, K)
    U = L / (2.0 * Ksafe[:, None])  # [3,3] tailored directions

    # basis quadratic parts: M[s] (3x3 sym) for s=0..8
    E = np.eye(3, dtype=np.float64)
    dirs = [
        (E[0], E[0]), (E[1], E[1]), (E[2], E[2]),
        (E[0] + E[1], E[0] + E[1]),
        (E[0] + E[2], E[0] + E[2]),
        (E[1] + E[2], E[1] + E[2]),
    ]
    M = np.zeros((9, 3, 3))
    for s, (u, v) in enumerate(dirs):
        M[s] = np.outer(u, v)
    for c in range(3):
        M[6 + c] = np.outer(U[c], U[c])

    def sym6(S):
        return np.array(
            [S[0, 0], S[1, 1], S[2, 2], S[0, 1] + S[1, 0],
             S[0, 2] + S[2, 0], S[1, 2] + S[2, 1]]
        )

    B6 = np.stack([sym6(M[s]) for s in range(9)])  # [9,6]
    W = np.zeros((3, 9))
    for c in range(3):
        rhs = sym6(Q[c]) - Ksafe[c] * B6[6 + c]
        W[c, :6] = np.linalg.solve(B6[:6].T, rhs)
        W[c, 6 + c] = Ksafe[c]
    kconst = K - Ksafe
    return U, W, kconst


def _host_tensors(a: np.ndarray):
    """pv [106,127] bf16, ab [127,42] f32, bias [127,1] f32.

    mm1 column layout (M=127): col 9g+s = form s of group g; col 126 is the
    constant generator (V=0, bias 1 -> R=1). Tailored forms (s=6+c) get
    bias 1 so R = (u_c.x + 1)^2. pv rows duplicated at bases 0 and 64.
    """
    U, W, kconst = _coeffs(a.astype(np.float64))
    pv1 = np.zeros((42, 127), dtype=np.float32)
    bias = np.zeros((127, 1), dtype=np.float32)
    ab = np.zeros((127, 42), dtype=np.float32)
    forms = [
        [(0, 1.0)], [(1, 1.0)], [(2, 1.0)],
        [(0, 1.0), (1, 1.0)], [(0, 1.0), (2, 1.0)], [(1, 1.0), (2, 1.0)],
    ]
    for g in range(G):
        for s in range(9):
            col = 9 * g + s
            if s < 6:
                for j, v in forms[s]:
                    pv1[3 * g + j, col] = v
            else:
                c = s - 6
                for j in range(3):
                    pv1[3 * g + j, col] = U[c, j]
                bias[col, 0] = 1.0
        for c in range(3):
            orow = 3 * g + c
            for s in range(9):
                ab[9 * g + s, orow] = W[c, s]
            ab[126, orow] = kconst[c]
    bias[126, 0] = 1.0
    pv = np.zeros((106, 127), dtype=np.float32)
    pv[0:42] = pv1
    pv[64:106] = pv1
    return pv.astype(ml_dtypes.bfloat16), ab.astype(ml_dtypes.bfloat16), bias


def _pack_x(x_core: np.ndarray) -> np.ndarray:
    """[N_PER_CORE, 3] f32 -> Z2 [106, 4608] bf16 (device input layout)."""
    xp = np.zeros((NPAD, 3), dtype=np.float32)
    xp[:N_PER_CORE] = x_core
    z = xp.reshape(F, G, 3).transpose(1, 2, 0).reshape(42, F)
    z2 = np.zeros((106, HCOLS), dtype=np.float32)
    z2[0:42] = z[:, :HCOLS]
    z2[64:106] = z[:, HCOLS:]
    return np.ascontiguousarray(z2.astype(ml_dtypes.bfloat16))


def _unpack_o(o: np.ndarray) -> np.ndarray:
    """o [106, 4608] bf16 -> [N_PER_CORE, 3] f32."""
    of = np.asarray(o, dtype=np.float32)
    full = np.empty((NPAD, 3), dtype=np.float32)
    # block k: rows 64*(k%2)+3g+cc, cols 512*(k//2)+u of o hold point
    # p = 14*m+g, m = 4608*h + 512*j + u, h = (k//3)%2, j = 3*(k//6)+k%3
    for k in range(18):
        v, i = k // 3, k % 3
        h, j = v % 2, 3 * (v // 2) + i
        p, s = k // 2, k % 2
        blk = of[64 * s : 64 * s + 42, CH * p : CH * (p + 1)]  # [3g+cc, u]
        m0 = 4608 * h + 512 * j
        full[G * m0 : G * (m0 + CH)] = (
            blk.reshape(G, 3, CH).transpose(2, 0, 1).reshape(G * CH, 3)
        )
    return full[:N_PER_CORE]


def kernel(x: np.ndarray, a: np.ndarray) -> np.ndarray:
    x = np.ascontiguousarray(x, dtype=np.float32)
    a = np.ascontiguousarray(a, dtype=np.float32)
    if "nc" not in _CACHE:
        _CACHE["nc"] = _build_nc()
    nc = _CACHE["nc"]

    pv, ab, bias = _host_tensors(a)
    in_maps = []
    for ci in range(N_CORES):
        z2 = _pack_x(x[ci * N_PER_CORE : (ci + 1) * N_PER_CORE])
        in_maps.append({"z": z2, "pv": pv, "ab": ab, "bias": bias})

    res = run_bass_kernel_spmd(nc, in_maps, list(range(N_CORES)))

    out = np.empty((N_POINTS, 3), dtype=np.float32)
    for ci in range(N_CORES):
        out[ci * N_PER_CORE : (ci + 1) * N_PER_CORE] = _unpack_o(
            res.results[ci]["o"]
        )
    return out


# revision 11
# speedup vs baseline: 1.8652x; 1.6683x over previous
"""Trainium2 Bass kernel for nn_Grid_fun: out = tile(feat(z), 6) @ a.

Math: z = [x, 1] (N,4); feat = (z otimes z).reshape(N,16); out = tile(feat,6) @ a
    = feat @ a_eff  where a_eff = a.reshape(6,16,3).sum(0)   [16,3]
    => out[n,c] = z[n]^T A_c z[n],  A_c = a_eff[:,c].reshape(4,4)

Device algorithm (per core, data-parallel over N):
  Host stages x as Z2 [106, 4608] bf16: half h at partition base 64h holds
  rows 3g+j of z-cols m = 4608h + u (G=14 points per z-col, F=9216 z-cols).
  PE base partitions must be in {0,64} for K,M<=64 - hence the two-half
  layout (~83% DMA partition utilization, bf16 halves the line bytes).
  Per vps tile v (6 total; h=v%2, col-group jg=v//2):
    mm1 x3: V[127, 512i..] = pv^T @ Z2[64h:64h+42, 512(3jg+i)..]   (bf16)
    ACT:    R[127, 1536] = Square(V + bias)  (canonical/tailored square
            basis; R row 126 = (0+1)^2 = 1 carries the constant via ab)
    mm2 x3: block k=3v+i -> pps[k//2][64*(k%2) : +42] = ab^T @ R  (fp32r,
            2-stacked in PSUM at offsets {0,64})
  DVE copies pps -> bf16 SBUF (junk rows 42:64 ignored by host), gpsimd
  issues the output DMAs. PE warm-up matmuls fight the HAM clock gate.
"""

import sys

if "/opt/trn_rl_repo" not in sys.path:
    sys.path.insert(0, "/opt/trn_rl_repo")

from contextlib import ExitStack

import ml_dtypes
import numpy as np

import concourse.bass as bass
import concourse.mybir as mybir
import concourse.tile as tile
from concourse import bacc
from concourse.bass_utils import run_bass_kernel_spmd

N_CORES = 8
N_POINTS = 1_000_000
N_PER_CORE = N_POINTS // N_CORES  # 125000
G = 14  # points per z-column
F = 9216  # z-columns per core (18*512); G*F = 129024 >= N_PER_CORE
NPAD = G * F
HCOLS = F // 2  # 4608 z-cols per half
NV = 6  # vps tiles (3 blocks of 512 each)
CH = 512
N_WARM = 12  # PE warm-up matmuls

_CACHE: dict = {}


def _build_nc():
    nc = bacc.Bacc("TRN2", target_bir_lowering=False)
    f32 = mybir.dt.float32
    f32r = mybir.dt.float32r
    bf16 = mybir.dt.bfloat16

    # All DMA partition counts are multiples of 16: the DGE splits one
    # DMA's descriptors into equal chunks over the largest divisor of
    # ndesc <= 16 SDMA engines (106 rows -> 2 engines; 112 -> 16).
    z_d = nc.dram_tensor("z", [112, HCOLS], bf16, kind="ExternalInput")
    pv_d = nc.dram_tensor("pv", [112, 127], bf16, kind="ExternalInput")
    ab_d = nc.dram_tensor("ab", [128, 42], bf16, kind="ExternalInput")
    bias_d = nc.dram_tensor("bias", [128, 1], f32, kind="ExternalInput")
    o_d = nc.dram_tensor("o", [112, HCOLS], bf16, kind="ExternalOutput")
    sink_d = nc.dram_tensor("sink", [128, 2], f32, kind="ExternalOutput")

    with tile.TileContext(nc) as tc:
        with ExitStack() as ctx:
            cpool = ctx.enter_context(tc.tile_pool(name="consts", bufs=1))
            zpool = ctx.enter_context(tc.tile_pool(name="zt", bufs=2))
            rpool = ctx.enter_context(tc.tile_pool(name="rt", bufs=2))
            opool = ctx.enter_context(tc.tile_pool(name="ot", bufs=2))
            vpool = ctx.enter_context(
                tc.tile_pool(name="vps", bufs=2, space="PSUM")
            )
            ppool = ctx.enter_context(
                tc.tile_pool(name="pps", bufs=2, space="PSUM")
            )

            pv = cpool.tile([112, 127], bf16)
            nc.sync.dma_start(pv[:], pv_d[:, :])
            ab = cpool.tile([128, 42], bf16)
            nc.sync.dma_start(ab[:], ab_d[:, :])
            bias = cpool.tile([128, 1], f32)
            nc.sync.dma_start(bias[:], bias_d[:, :])

            # ACT table warm-up (Square table load ~1.3us) off critical path:
            # depends only on the tiny bias DMA, so it runs at ~0.3us.
            wsq = cpool.tile([128, 2], f32)
            nc.scalar.activation(
                wsq[:, 0:1], bias[:], mybir.ActivationFunctionType.Square,
                bias=bias[:], scale=1.0,
            )
            # PE warm-up: HAM un-throttles after ~3.4us of sustained PE
            # activity; burn the initial DMA-wait advancing that window.
            warm = ppool.tile([127, CH], f32, tag="pps")
            for _ in range(N_WARM):
                nc.tensor.matmul(
                    warm[:, 0:127], pv[0:42, :], pv[0:42, 0:127],
                    start=True, stop=True,
                )
            # Keep the warm-ups live (read one column into the sink output).
            nc.vector.tensor_copy(wsq[0:127, 1:2], warm[:, 0:1])
            nc.sync.dma_start(sink_d[:, :], wsq[:])

            pps_tiles = {}
            for v in range(NV):
                h, jg = v % 2, v // 2
                if h == 0:
                    zt = zpool.tile([112, 3 * CH], bf16)
                    nc.sync.dma_start(
                        zt[:], z_d[:, 3 * CH * jg : 3 * CH * (jg + 1)]
                    )

                vps = vpool.tile([127, 3 * CH], f32)
                for i in range(3):
                    nc.tensor.matmul(
                        vps[:, i * CH : (i + 1) * CH],
                        pv[64 * h : 64 * h + 42, :],
                        zt[64 * h : 64 * h + 42, i * CH : (i + 1) * CH],
                        start=True,
                        stop=True,
                    )

                rt = rpool.tile([127, 3 * CH], bf16)
                nc.scalar.activation(
                    rt[:],
                    vps[:],
                    mybir.ActivationFunctionType.Square,
                    bias=bias[0:127],
                    scale=1.0,
                )
                for i in range(3):
                    k = 3 * v + i
                    p, s = k // 2, k % 2
                    if s == 0:
                        pps_tiles[p] = ppool.tile(
                            [106, CH], f32, tag="pps", name=f"pps{p}"
                        )
                    nc.tensor.matmul(
                        pps_tiles[p][64 * s : 64 * s + 42, :],
                        ab[0:127, :],
                        rt[:, i * CH : (i + 1) * CH],
                        start=True,
                        stop=True,
                    )
                    if s == 1:
                        if p % 3 == 0:
                            ot = opool.tile([112, 3 * CH], bf16)
                        nc.vector.tensor_copy(
                            ot[0:106, (p % 3) * CH : (p % 3 + 1) * CH],
                            pps_tiles[p][:],
                        )
                        if p % 3 == 2:
                            oc = (p - 2) * CH
                            nc.sync.dma_start(
                                o_d[:, oc : oc + 3 * CH], ot[:]
                            )
    nc.compile()
    return nc


def _coeffs(a: np.ndarray):
    """Host-side prep of the constant matrices from param a [96,3]."""
    a_eff = a.reshape(6, 16, 3).sum(0)  # [16,3]
    A = a_eff.T.reshape(3, 4, 4)  # A[c] with out_c = z^T A_c z
    As = 0.5 * (A + A.transpose(0, 2, 1))  # symmetrize
    Q = As[:, :3, :3]  # [3,3,3] quadratic part
    L = 2.0 * As[:, :3, 3]  # [3,3] linear coefs
    K = As[:, 3, 3].copy()  # [3] constants
    # guard tiny K (u_c = L_c / (2 K_c)); shift the constant via kconst fold
    Ksafe = np.where(np.abs(K) < 1e-3, 1.0, K)
    U = L / (2.0 * Ksafe[:, None])  # [3,3] tailored directions

    # basis quadratic parts: M[s] (3x3 sym) for s=0..8
    E = np.eye(3, dtype=np.float64)
    dirs = [
        (E[0], E[0]), (E[1], E[1]), (E[2], E[2]),
        (E[0] + E[1], E[0] + E[1]),
        (E[0] + E[2], E[0] + E[2]),
        (E[1] + E[2], E[1] + E[2]),
    ]
    M = np.zeros((9, 3, 3))
    for s, (u, v) in enumerate(dirs):
        M[s] = np.outer(u, v)
    for c in range(3):
        M[6 + c] = np.outer(U[c], U[c])

    def sym6(S):
        return np.array(
            [S[0, 0], S[1, 1], S[2, 2], S[0, 1] + S[1, 0],
             S[0, 2] + S[2, 0], S[1, 2] + S[2, 1]]
        )

    B6 = np.stack([sym6(M[s]) for s in range(9)])  # [9,6]
    W = np.zeros((3, 9))
    for c in range(3):
        rhs = sym6(Q[c]) - Ksafe[c] * B6[6 + c]
        W[c, :6] = np.linalg.solve(B6[:6].T, rhs)
        W[c, 6 + c] = Ksafe[c]
    kconst = K - Ksafe
    return U, W, kconst


def _host_tensors(a: np.ndarray):
    """pv [112,127] bf16, ab [128,42] bf16, bias [128,1] f32.

    mm1 column layout (M=127): col 9g+s = form s of group g; col 126 is the
    constant generator (V=0, bias 1 -> R=1). Tailored forms (s=6+c) get
    bias 1 so R = (u_c.x + 1)^2. pv rows duplicated at bases 0 and 64.
    """
    U, W, kconst = _coeffs(a.astype(np.float64))
    pv1 = np.zeros((42, 127), dtype=np.float32)
    bias = np.zeros((128, 1), dtype=np.float32)
    ab = np.zeros((128, 42), dtype=np.float32)
    forms = [
        [(0, 1.0)], [(1, 1.0)], [(2, 1.0)],
        [(0, 1.0), (1, 1.0)], [(0, 1.0), (2, 1.0)], [(1, 1.0), (2, 1.0)],
    ]
    for g in range(G):
        for s in range(9):
            col = 9 * g + s
            if s < 6:
                for j, v in forms[s]:
                    pv1[3 * g + j, col] = v
            else:
                c = s - 6
                for j in range(3):
                    pv1[3 * g + j, col] = U[c, j]
                bias[col, 0] = 1.0
        for c in range(3):
            orow = 3 * g + c
            for s in range(9):
                ab[9 * g + s, orow] = W[c, s]
            ab[126, orow] = kconst[c]
    bias[126, 0] = 1.0
    pv = np.zeros((112, 127), dtype=np.float32)
    pv[0:42] = pv1
    pv[64:106] = pv1
    return pv.astype(ml_dtypes.bfloat16), ab.astype(ml_dtypes.bfloat16), bias


def _pack_x(x_core: np.ndarray) -> np.ndarray:
    """[N_PER_CORE, 3] f32 -> Z2 [112, 4608] bf16 (device input layout)."""
    xp = np.zeros((NPAD, 3), dtype=np.float32)
    xp[:N_PER_CORE] = x_core
    z = xp.reshape(F, G, 3).transpose(1, 2, 0).reshape(42, F)
    z2 = np.zeros((112, HCOLS), dtype=np.float32)
    z2[0:42] = z[:, :HCOLS]
    z2[64:106] = z[:, HCOLS:]
    return np.ascontiguousarray(z2.astype(ml_dtypes.bfloat16))


def _unpack_o(o: np.ndarray) -> np.ndarray:
    """o [112, 4608] bf16 -> [N_PER_CORE, 3] f32."""
    of = np.asarray(o, dtype=np.float32)
    full = np.empty((NPAD, 3), dtype=np.float32)
    # block k: rows 64*(k%2)+3g+cc, cols 512*(k//2)+u of o hold point
    # p = 14*m+g, m = 4608*h + 512*j + u, h = (k//3)%2, j = 3*(k//6)+k%3
    for k in range(18):
        v, i = k // 3, k % 3
        h, j = v % 2, 3 * (v // 2) + i
        p, s = k // 2, k % 2
        blk = of[64 * s : 64 * s + 42, CH * p : CH * (p + 1)]  # [3g+cc, u]
        m0 = 4608 * h + 512 * j
        full[G * m0 : G * (m0 + CH)] = (
            blk.reshape(G, 3, CH).transpose(2, 0, 1).reshape(G * CH, 3)
        )
    return full[:N_PER_CORE]


def kernel(x: np.ndarray, a: np.ndarray) -> np.ndarray:
    x = np.ascontiguousarray(x, dtype=np.float32)
    a = np.ascontiguousarray(a, dtype=np.float32)
    if "nc" not in _CACHE:
        _CACHE["nc"] = _build_nc()
    nc = _CACHE["nc"]

    pv, ab, bias = _host_tensors(a)
    in_maps = []
    for ci in range(N_CORES):
        z2 = _pack_x(x[ci * N_PER_CORE : (ci + 1) * N_PER_CORE])
        in_maps.append({"z": z2, "pv": pv, "ab": ab, "bias": bias})

    res = run_bass_kernel_spmd(nc, in_maps, list(range(N_CORES)))

    out = np.empty((N_POINTS, 3), dtype=np.float32)
    for ci in range(N_CORES):
        out[ci * N_PER_CORE : (ci + 1) * N_PER_CORE] = _unpack_o(
            res.results[ci]["o"]
        )
    return out
